# revision 25
# baseline (speedup 1.0000x reference)
"""AdaptiveRankLinear on Trainium2, 8-core data-parallel Bass/Tile kernel.

Computes  y = x + gamma * (((rmsnorm(x) * norm_weight) @ U) * (S*keep)) @ V
with keep = adaptive-rank mask from the singular-value energy of S.

Sharding: x is flattened to [8192, 4096] tokens and split into 8 shards of
1024 tokens (one per NeuronCore); U/S/V/norm_weight/gamma are tiny and
replicated (folded host-side into two small matrices).

v9: d-major ("transposed") data layout chosen host-side.  Each 128-token
group is stored as [128 dims-of-block-j, (j, token)] so the contraction
over D needs NO on-chip transposes and NO PSUM->SBUF evacuations:
  hT[strip] += U2_j.T @ x_j     directly (x_j = [128 dims, 128 tokens])
  sumsq_row  = sum_j ones.T @ square(x_j)   (PE column-sum of ACT square)
  rstd_row   = recip(sqrt(...)) on a [1,128] row
  rb         = ones_col.T @ rstd_row        (K=1 rank-1 PE broadcast)
  hs         = hT * rb  (rstd folded here, one tiny DVE op)
  delta_j    = V2r_j.T @ hs  -> psum [128 dims, 128 tokens]
  y_j        = x_j + delta_j (plain tensor adds: 3 DVE + 1 ACT+GpSimd)
Host un-permutes the output.  DMA shapes are identical to the token-major
version (contiguous [128 x 8KB] tiles).
"""
import ml_dtypes
import numpy as np

import concourse.bass as bass
import concourse.tile as tile
from concourse import mybir
from concourse.bass_utils import run_bass_kernel_spmd
from concourse.vector_clock import ScopedClock

# ----------------------------------------------------------------------------
# Workaround: this container's walrus accepts at most ONE sync wait per
# instruction, while Tile's sem-assigner can attach several.  Split extras
# into engine-local no-ops placed immediately before the over-waited
# instruction; same for the kernel-tail drain.
# ----------------------------------------------------------------------------
_MAXW = 1


def _split_bb_waits(nc, bb):
    insts = list(bb.instructions)
    out = []
    changed = False
    for inst in insts:
        si = inst.sync_info
        if si is not None and len(si.on_wait) > _MAXW:
            changed = True
            waits = list(si.on_wait)
            extra, keep = waits[:-_MAXW], waits[-_MAXW:]
            for k, w in enumerate(extra):
                nop = mybir.InstNoOp(name=f"{inst.name}_wsplit{k}", ins=[],
                                     outs=[])
                nop.engine = inst.engine
                nop.sync_info = mybir.SyncInfo(on_wait=[w], on_update=[])
                nc.register_instruction(nop, overwrite=True)
                out.append(nop)
            inst.sync_info = mybir.SyncInfo(on_wait=keep,
                                            on_update=list(si.on_update))
        out.append(inst)
    if changed:
        bb.instructions = out


def _patched_drain_and_barrier(self, tick_clock, wait_clock):
    for f in self.nc.m.functions:
        for bb in f.blocks:
            _split_bb_waits(self.nc, bb)

    drain_inst = self.nc.sync.drain()
    wait_clock.add_sem_waits(
        drain_inst.ins, ScopedClock({None: tick_clock.global_clock})
    )
    si = drain_inst.ins.sync_info
    if si is not None and len(si.on_wait) > _MAXW:
        waits = list(si.on_wait)
        drain_inst.ins.sync_info = mybir.SyncInfo(
            on_wait=waits[:_MAXW], on_update=list(si.on_update)
        )
        rest = waits[_MAXW:]
        for i in range(0, len(rest), _MAXW):
            nop = self.nc.sync.nop(nofuse=True, hint="drain_wait_spill")
            nop.ins.sync_info = mybir.SyncInfo(
                on_wait=rest[i:i + _MAXW], on_update=[]
            )

    self.nc.all_engine_barrier()
    assert self.sems is not None
    popped = self.nc._tile_sem_poison_stack.pop()
    assert popped is self._sem_poison
    self.nc.clear_and_free_semaphores(list(self.sems.allocated().values()))
    self.nc.all_engine_barrier()


tile.TileContext._drain_and_barrier = _patched_drain_and_barrier

# ----------------------------------------------------------------------------
# Problem constants (hardcoded; kernel.py must be self-contained).
# ----------------------------------------------------------------------------
N_CORES = 8
B, T, D = 4, 2048, 4096
TOK = B * T              # 8192
R = 16
SHARD = TOK // N_CORES   # 1024
PT = 128                 # tokens per group
NT = SHARD // PT         # 8
KB = D // 128            # 32 contraction blocks
EPS = 1e-6
ENERGY_THRESHOLD = 0.95
F32 = mybir.dt.float32
BF16 = mybir.dt.bfloat16
NP_BF16 = ml_dtypes.bfloat16
AF = mybir.ActivationFunctionType
ALU = mybir.AluOpType

SB = 8                   # j-blocks per PE sub-block
NSB = KB // SB           # 4 sub-blocks per group
CW = 1024                # y/delta chunk width (8 j-blocks, two f32 banks)
NC_CH = D // CW          # 4 chunks per group
ACT_Y = (3,)             # y-chunk routed via ACT copy + GpSimd adds
RP = 32                  # U ranks padded to one 32-col PE strip
NSTRIP = 4               # concurrent col-tiled U strips


def build_nc():
    nc = bass.Bass("TRN2", target_bir_lowering=False, debug=False,
                   num_devices=N_CORES)
    x = nc.declare_dram_parameter("x", [SHARD, D], BF16, isOutput=False)
    u = nc.declare_dram_parameter("u", [128, KB * RP], BF16, isOutput=False)
    v = nc.declare_dram_parameter("v", [128, D], BF16, isOutput=False)
    out = nc.declare_dram_parameter("out", [SHARD, D], BF16, isOutput=True)

    with tile.TileContext(nc) as tc:
        with (
            tc.tile_pool(name="singles", bufs=1) as singles,
            tc.tile_pool(name="xin", bufs=8) as xin,
            tc.tile_pool(name="sqb", bufs=2) as sqb,
            tc.tile_pool(name="yout", bufs=5) as yout,
            tc.tile_pool(name="smalls", bufs=4) as smalls,
            tc.tile_pool(name="keeps", bufs=3) as keeps,
            tc.tile_pool(name="scratch", bufs=2) as scratch,
            tc.tile_pool(name="ss_ps", bufs=1, space="PSUM") as ss_ps,
            tc.tile_pool(name="h_ps", bufs=2, space="PSUM") as h_ps,
            tc.tile_pool(name="rb_ps", bufs=1, space="PSUM") as rb_ps,
            tc.tile_pool(name="d_ps", bufs=2, space="PSUM") as d_ps,
        ):
            u_sb = singles.tile([128, KB, RP], BF16)
            v_sb = singles.tile([128, D], BF16)
            ones_d = singles.tile([128, 1], BF16)
            nc.vector.memset(ones_d, 1.0)
            one_row = singles.tile([1, PT], BF16)
            nc.vector.memset(one_row, 1.0)
            eps_sb = singles.tile([1, 1], F32)
            nc.vector.memset(eps_sb, EPS)

            prev = None  # expand-state of group i-1

            def emit_expand_chunks(st, count):
                """Emit `count` (8 delta-MMs + y) chunks of a pending group."""
                if st is None:
                    return
                if st["y_sb"] is None:
                    st["y_sb"] = yout.tile([PT, D], BF16, name="y_sb",
                                           tag="y_sb")
                y_sb, x_sb, t0 = st["y_sb"], st["x_sb"], st["t0"]
                for _ in range(count):
                    n = st["n"]
                    if n >= NC_CH:
                        return
                    st["n"] = n + 1
                    dps = d_ps.tile([128, CW], F32, tag="d")
                    for q in range(SB):
                        j = n * SB + q
                        nc.tensor.matmul(
                            out=dps[:, q * PT:(q + 1) * PT],
                            lhsT=v_sb[:, j * PT:(j + 1) * PT],
                            rhs=st["hs_sb"],
                            start=True, stop=True)
                    c0 = n * CW
                    ysl = y_sb[:, c0:c0 + CW]
                    xsl = x_sb[:, c0:c0 + CW]
                    if n in ACT_Y and not st.get("final"):
                        dsb = scratch.tile([128, CW], BF16, tag="dsb")
                        nc.scalar.copy(out=dsb, in_=dps)
                        hw = CW // 2
                        nc.gpsimd.tensor_add(out=ysl[:, :hw],
                                             in0=dsb[:, :hw],
                                             in1=xsl[:, :hw])
                        nc.gpsimd.tensor_add(out=ysl[:, hw:],
                                             in0=dsb[:, hw:],
                                             in1=xsl[:, hw:])
                    else:
                        nc.vector.tensor_add(out=ysl, in0=dps, in1=xsl)
                    if n == NC_CH // 2 - 1:
                        nc.gpsimd.dma_start(out=out[t0:t0 + PT, :D // 2],
                                            in_=y_sb[:, :D // 2])
                    elif n == NC_CH - 1:
                        nc.gpsimd.dma_start(out=out[t0:t0 + PT, D // 2:],
                                            in_=y_sb[:, D // 2:])

            for it in range(NT):
                t0 = it * PT
                x_sb = xin.tile([PT, D], BF16, tag="x_sb")
                if it == 0:
                    # quarter-DMAs so the first h-MMs start early; weights
                    # go out after the first group so it is not delayed.
                    for qd in range(4):
                        nc.sync.dma_start(
                            out=x_sb[:, qd * (D // 4):(qd + 1) * (D // 4)],
                            in_=x[t0:t0 + PT,
                                  qd * (D // 4):(qd + 1) * (D // 4)])
                    nc.sync.dma_start(
                        out=u_sb, in_=u.rearrange("p (k r) -> p k r", r=RP))
                    nc.sync.dma_start(out=v_sb, in_=v[:, :])
                else:
                    nc.sync.dma_start(out=x_sb, in_=x[t0:t0 + PT, :])

                sq = sqb.tile([PT, D], BF16, tag="sq")
                h_psum = h_ps.tile([128, PT], F32, tag="h")
                # expands of group i-1 and h-MMs first: neither waits on
                # this group's full x, so the PE/ACT queues never stall on
                # the input stream ramp.
                for g in range(NSB):
                    for q in range(SB):
                        j = g * SB + q
                        c = j % NSTRIP
                        nc.tensor.matmul(
                            out=h_psum[32 * c:32 * (c + 1), :],
                            lhsT=u_sb[:, j, :],
                            rhs=x_sb[:, j * PT:(j + 1) * PT],
                            start=(j // NSTRIP == 0),
                            stop=(j // NSTRIP == KB // NSTRIP - 1),
                            tile_position=(0, 32 * c),
                            skip_group_check=True)
                    emit_expand_chunks(prev, 1)
                emit_expand_chunks(prev, NC_CH)  # flush any remainder

                # square (group 0: per-quarter, pipelined with its DMAs)
                nsq = 4 if it == 0 else 1
                for s in range(nsq):
                    w = D // nsq
                    nc.scalar.activation(out=sq[:, s * w:(s + 1) * w],
                                         in_=x_sb[:, s * w:(s + 1) * w],
                                         func=AF.Square)
                ss = ss_ps.tile([1, PT], F32, tag="ss")
                for j in range(KB):
                    nc.tensor.matmul(
                        out=ss,
                        lhsT=ones_d,
                        rhs=sq[:, j * PT:(j + 1) * PT],
                        start=(j == 0), stop=(j == KB - 1),
                        skip_group_check=True)

                # rstd: sqrt row -> rank-1 PE broadcast -> recip on 128
                # partitions (much cheaper than recip on a 1-row tile)
                std_row = smalls.tile([1, PT], BF16, tag="std")
                with nc.allow_low_precision(
                        reason="rstd scales a 1e-5-weighted delta"):
                    nc.scalar.activation(out=std_row, in_=ss, func=AF.Sqrt,
                                         bias=eps_sb, scale=1.0 / D)
                    rb = rb_ps.tile([128, PT], F32, tag="rb")
                    nc.tensor.matmul(out=rb, lhsT=one_row, rhs=std_row,
                                     start=True, stop=True)
                    rb_sb = keeps.tile([128, PT], F32, tag="rb_sb")
                    nc.vector.reciprocal(out=rb_sb, in_=rb)
                hs_sb = keeps.tile([128, PT], BF16, tag="hs")
                nc.vector.tensor_mul(out=hs_sb, in0=h_psum, in1=rb_sb)

                prev = {"hs_sb": hs_sb, "x_sb": x_sb, "t0": t0,
                        "y_sb": None, "n": 0,
                        "final": it == NT - 1}

            emit_expand_chunks(prev, NC_CH)
    return nc


def _rank_mask_np(S):
    s_abs = np.abs(S)
    cum = np.cumsum(s_abs) / max(float(s_abs.sum()), 1e-8)
    hit = cum >= ENERGY_THRESHOLD
    r = int(np.argmax(hit)) + 1 if hit.any() else S.shape[0]
    return (np.arange(S.shape[0]) < r).astype(S.dtype)


def _permute_dmaj(xs):
    """[1024, 4096] token-major -> d-major group layout, same shape.

    out[g*128 + p, j*128 + c] = xs[g*128 + c, j*128 + p]
    """
    xr = xs.reshape(NT, PT, KB, 128)          # [g, c, j, p]
    return np.ascontiguousarray(
        xr.transpose(0, 3, 2, 1)).reshape(SHARD, D)


def _unpermute_dmaj(ys):
    """Inverse of _permute_dmaj (it is an involution up to axis names)."""
    yr = ys.reshape(NT, 128, KB, PT)          # [g, p, j, c]
    return np.ascontiguousarray(
        yr.transpose(0, 3, 2, 1)).reshape(SHARD, D)


def make_in_maps(x, U, S, V, norm_weight, gamma):
    S = np.asarray(S, dtype=np.float32)
    keep = _rank_mask_np(S)
    U2 = (np.asarray(norm_weight, dtype=np.float32)[:, None]
          * np.asarray(U, dtype=np.float32)
          * (S * keep)[None, :]).astype(NP_BF16)
    U2p = np.zeros((D, RP), dtype=NP_BF16)
    U2p[:, :R] = U2
    U2p = np.ascontiguousarray(
        U2p.reshape(KB, 128, RP).transpose(1, 0, 2).reshape(128, KB * RP))
    V2 = (np.asarray(V, dtype=np.float32)
          * np.asarray(gamma, dtype=np.float32)[None, :]).astype(NP_BF16)
    V2r = np.zeros((128, D), dtype=NP_BF16)
    for c in range(NSTRIP):
        V2r[32 * c:32 * c + R, :] = V2
    xf = np.ascontiguousarray(
        np.asarray(x, dtype=np.float32).reshape(TOK, D)).astype(NP_BF16)
    shards = [_permute_dmaj(s) for s in np.split(xf, N_CORES, axis=0)]
    return [{"x": s, "u": U2p, "v": V2r} for s in shards]


_CACHED_NC = None


def run(x, U, S, V, norm_weight, gamma, trace=False, **kw):
    global _CACHED_NC
    if _CACHED_NC is None:
        _CACHED_NC = build_nc()
    in_maps = make_in_maps(x, U, S, V, norm_weight, gamma)
    res = run_bass_kernel_spmd(_CACHED_NC, in_maps,
                               core_ids=list(range(N_CORES)), trace=trace,
                               **kw)
    outs = [_unpermute_dmaj(np.asarray(res.results[i]["out"]))
            for i in range(N_CORES)]
    y = np.concatenate(outs, axis=0).reshape(B, T, D).astype(np.float32)
    return y, res


def kernel(x, U, S, V, norm_weight, gamma):
    y, _ = run(x, U, S, V, norm_weight, gamma, trace=False)
    return y


# revision 26
# speedup vs baseline: 1.1647x; 1.1647x over previous
"""AdaptiveRankLinear on Trainium2, 8-core data-parallel Bass/Tile kernel.

Computes  y = x + gamma * (((rmsnorm(x) * norm_weight) @ U) * (S*keep)) @ V
with keep = adaptive-rank mask from the singular-value energy of S.

Sharding: x is flattened to [8192, 4096] tokens and split into 8 shards of
1024 tokens (one per NeuronCore); U/S/V/norm_weight/gamma are tiny and
replicated (folded host-side into two small matrices).

v9: d-major ("transposed") data layout chosen host-side.  Each 128-token
group is stored as [128 dims-of-block-j, (j, token)] so the contraction
over D needs NO on-chip transposes and NO PSUM->SBUF evacuations:
  hT[strip] += U2_j.T @ x_j     directly (x_j = [128 dims, 128 tokens])
  sumsq_row  = sum_j ones.T @ square(x_j)   (PE column-sum of ACT square)
  rstd_row   = recip(sqrt(...)) on a [1,128] row
  rb         = ones_col.T @ rstd_row        (K=1 rank-1 PE broadcast)
  hs         = hT * rb  (rstd folded here, one tiny DVE op)
  delta_j    = V2r_j.T @ hs  -> psum [128 dims, 128 tokens]
  y_j        = x_j + delta_j (plain tensor adds: 3 DVE + 1 ACT+GpSimd)
Host un-permutes the output.  DMA shapes are identical to the token-major
version (contiguous [128 x 8KB] tiles).
"""
import ml_dtypes
import numpy as np

import concourse.bass as bass
import concourse.tile as tile
from concourse import mybir
from concourse.bass_utils import run_bass_kernel_spmd
from concourse.vector_clock import ScopedClock

# ----------------------------------------------------------------------------
# Workaround: this container's walrus accepts at most ONE sync wait per
# instruction, while Tile's sem-assigner can attach several.  Split extras
# into engine-local no-ops placed immediately before the over-waited
# instruction; same for the kernel-tail drain.
# ----------------------------------------------------------------------------
_MAXW = 1


def _split_bb_waits(nc, bb):
    insts = list(bb.instructions)
    out = []
    changed = False
    for inst in insts:
        si = inst.sync_info
        if si is not None and len(si.on_wait) > _MAXW:
            changed = True
            waits = list(si.on_wait)
            extra, keep = waits[:-_MAXW], waits[-_MAXW:]
            for k, w in enumerate(extra):
                nop = mybir.InstNoOp(name=f"{inst.name}_wsplit{k}", ins=[],
                                     outs=[])
                nop.engine = inst.engine
                nop.sync_info = mybir.SyncInfo(on_wait=[w], on_update=[])
                nc.register_instruction(nop, overwrite=True)
                out.append(nop)
            inst.sync_info = mybir.SyncInfo(on_wait=keep,
                                            on_update=list(si.on_update))
        out.append(inst)
    if changed:
        bb.instructions = out


def _patched_drain_and_barrier(self, tick_clock, wait_clock):
    for f in self.nc.m.functions:
        for bb in f.blocks:
            _split_bb_waits(self.nc, bb)

    drain_inst = self.nc.sync.drain()
    wait_clock.add_sem_waits(
        drain_inst.ins, ScopedClock({None: tick_clock.global_clock})
    )
    si = drain_inst.ins.sync_info
    if si is not None and len(si.on_wait) > _MAXW:
        waits = list(si.on_wait)
        drain_inst.ins.sync_info = mybir.SyncInfo(
            on_wait=waits[:_MAXW], on_update=list(si.on_update)
        )
        rest = waits[_MAXW:]
        for i in range(0, len(rest), _MAXW):
            nop = self.nc.sync.nop(nofuse=True, hint="drain_wait_spill")
            nop.ins.sync_info = mybir.SyncInfo(
                on_wait=rest[i:i + _MAXW], on_update=[]
            )

    self.nc.all_engine_barrier()
    assert self.sems is not None
    popped = self.nc._tile_sem_poison_stack.pop()
    assert popped is self._sem_poison
    self.nc.clear_and_free_semaphores(list(self.sems.allocated().values()))
    self.nc.all_engine_barrier()


tile.TileContext._drain_and_barrier = _patched_drain_and_barrier

# ----------------------------------------------------------------------------
# Problem constants (hardcoded; kernel.py must be self-contained).
# ----------------------------------------------------------------------------
N_CORES = 8
B, T, D = 4, 2048, 4096
TOK = B * T              # 8192
R = 16
SHARD = TOK // N_CORES   # 1024
PT = 128                 # tokens per group
NT = SHARD // PT         # 8
KB = D // 128            # 32 contraction blocks
EPS = 1e-6
ENERGY_THRESHOLD = 0.95
F32 = mybir.dt.float32
BF16 = mybir.dt.bfloat16
NP_BF16 = ml_dtypes.bfloat16
AF = mybir.ActivationFunctionType
ALU = mybir.AluOpType

SB = 8                   # j-blocks per PE sub-block
NSB = KB // SB           # 4 sub-blocks per group
CW = 1024                # y/delta chunk width (8 j-blocks, two f32 banks)
NC_CH = D // CW          # 4 chunks per group
ACT_Y = (3,)             # y-chunk routed via ACT copy + GpSimd adds
RP = 32                  # U ranks padded to one 32-col PE strip
NSTRIP = 4               # concurrent col-tiled U strips


def build_nc():
    nc = bass.Bass("TRN2", target_bir_lowering=False, debug=False,
                   num_devices=N_CORES)
    x = nc.declare_dram_parameter("x", [SHARD, D], BF16, isOutput=False)
    u = nc.declare_dram_parameter("u", [128, KB * RP], BF16, isOutput=False)
    v = nc.declare_dram_parameter("v", [128, D], BF16, isOutput=False)
    out = nc.declare_dram_parameter("out", [SHARD, D], BF16, isOutput=True)

    with tile.TileContext(nc) as tc:
        with (
            tc.tile_pool(name="singles", bufs=1) as singles,
            tc.tile_pool(name="xin", bufs=7) as xin,
            tc.tile_pool(name="sqb", bufs=2) as sqb,
            tc.tile_pool(name="yout", bufs=3) as yout,
            tc.tile_pool(name="smalls", bufs=4) as smalls,
            tc.tile_pool(name="keeps", bufs=3) as keeps,
            tc.tile_pool(name="scratch", bufs=2) as scratch,
            tc.tile_pool(name="ss_ps", bufs=1, space="PSUM") as ss_ps,
            tc.tile_pool(name="h_ps", bufs=2, space="PSUM") as h_ps,
            tc.tile_pool(name="rb_ps", bufs=1, space="PSUM") as rb_ps,
            tc.tile_pool(name="d_ps", bufs=2, space="PSUM") as d_ps,
        ):
            u_sb = singles.tile([128, KB, RP], BF16)
            v_sb = singles.tile([128, D], BF16)
            ones_d = singles.tile([128, 1], BF16)
            nc.vector.memset(ones_d, 1.0)
            one_row = singles.tile([1, PT], BF16)
            nc.vector.memset(one_row, 1.0)
            eps_sb = singles.tile([1, 1], F32)
            nc.vector.memset(eps_sb, EPS)

            prev = None  # expand-state of group i-1

            def emit_expand_chunks(st, count):
                """Emit `count` (8 delta-MMs + y) chunks of a pending group."""
                if st is None:
                    return
                if st["y_sb"] is None:
                    st["y_sb"] = yout.tile([PT, D], BF16, name="y_sb",
                                           tag="y_sb")
                y_sb, x_sb, t0 = st["y_sb"], st["x_sb"], st["t0"]
                for _ in range(count):
                    n = st["n"]
                    if n >= NC_CH:
                        return
                    st["n"] = n + 1
                    dps = d_ps.tile([128, CW], F32, tag="d")
                    for q in range(SB):
                        j = n * SB + q
                        nc.tensor.matmul(
                            out=dps[:, q * PT:(q + 1) * PT],
                            lhsT=v_sb[:, j * PT:(j + 1) * PT],
                            rhs=st["hs_sb"],
                            start=True, stop=True)
                    c0 = n * CW
                    ysl = y_sb[:, c0:c0 + CW]
                    xsl = x_sb[:, c0:c0 + CW]
                    if n in ACT_Y and not st.get("final"):
                        dsb = scratch.tile([128, CW], BF16, tag="dsb")
                        nc.scalar.copy(out=dsb, in_=dps)
                        hw = CW // 2
                        nc.gpsimd.tensor_add(out=ysl[:, :hw],
                                             in0=dsb[:, :hw],
                                             in1=xsl[:, :hw])
                        nc.gpsimd.tensor_add(out=ysl[:, hw:],
                                             in0=dsb[:, hw:],
                                             in1=xsl[:, hw:])
                    else:
                        nc.vector.tensor_add(out=ysl, in0=dps, in1=xsl)
                    if n == NC_CH // 2 - 1:
                        nc.gpsimd.dma_start(out=out[t0:t0 + PT, :D // 2],
                                            in_=y_sb[:, :D // 2])
                    elif n == NC_CH - 1:
                        nc.gpsimd.dma_start(out=out[t0:t0 + PT, D // 2:],
                                            in_=y_sb[:, D // 2:])

            for it in range(NT):
                t0 = it * PT
                x_sb = xin.tile([PT, D], BF16, tag="x_sb")
                if it == 0:
                    # quarter-DMAs so the first h-MMs start early; weights
                    # go out after the first group so it is not delayed.
                    for qd in range(4):
                        nc.sync.dma_start(
                            out=x_sb[:, qd * (D // 4):(qd + 1) * (D // 4)],
                            in_=x[t0:t0 + PT,
                                  qd * (D // 4):(qd + 1) * (D // 4)])
                    nc.sync.dma_start(
                        out=u_sb, in_=u.rearrange("p (k r) -> p k r", r=RP))
                    nc.sync.dma_start(out=v_sb, in_=v[:, :])
                else:
                    nc.sync.dma_start(out=x_sb, in_=x[t0:t0 + PT, :])

                sq = sqb.tile([PT, D], BF16, tag="sq")
                h_psum = h_ps.tile([128, PT], F32, tag="h")
                # expands of group i-1 and h-MMs first: neither waits on
                # this group's full x, so the PE/ACT queues never stall on
                # the input stream ramp.
                for g in range(NSB):
                    for q in range(SB):
                        j = g * SB + q
                        c = j % NSTRIP
                        nc.tensor.matmul(
                            out=h_psum[32 * c:32 * (c + 1), :],
                            lhsT=u_sb[:, j, :],
                            rhs=x_sb[:, j * PT:(j + 1) * PT],
                            start=(j // NSTRIP == 0),
                            stop=(j // NSTRIP == KB // NSTRIP - 1),
                            tile_position=(0, 32 * c),
                            skip_group_check=True)
                    emit_expand_chunks(prev, 1)
                emit_expand_chunks(prev, NC_CH)  # flush any remainder

                # square (group 0: per-quarter, pipelined with its DMAs)
                nsq = 4 if it == 0 else 1
                for s in range(nsq):
                    w = D // nsq
                    nc.scalar.activation(out=sq[:, s * w:(s + 1) * w],
                                         in_=x_sb[:, s * w:(s + 1) * w],
                                         func=AF.Square)
                ss = ss_ps.tile([1, PT], F32, tag="ss")
                for j in range(KB):
                    nc.tensor.matmul(
                        out=ss,
                        lhsT=ones_d,
                        rhs=sq[:, j * PT:(j + 1) * PT],
                        start=(j == 0), stop=(j == KB - 1),
                        skip_group_check=True)

                # rstd: sqrt row -> rank-1 PE broadcast -> recip on 128
                # partitions (much cheaper than recip on a 1-row tile)
                std_row = smalls.tile([1, PT], BF16, tag="std")
                with nc.allow_low_precision(
                        reason="rstd scales a 1e-5-weighted delta"):
                    nc.scalar.activation(out=std_row, in_=ss, func=AF.Sqrt,
                                         bias=eps_sb, scale=1.0 / D)
                    rb = rb_ps.tile([128, PT], F32, tag="rb")
                    nc.tensor.matmul(out=rb, lhsT=one_row, rhs=std_row,
                                     start=True, stop=True)
                    rb_sb = keeps.tile([128, PT], F32, tag="rb_sb")
                    nc.vector.reciprocal(out=rb_sb, in_=rb)
                hs_sb = keeps.tile([128, PT], BF16, tag="hs")
                nc.vector.tensor_mul(out=hs_sb, in0=h_psum, in1=rb_sb)

                prev = {"hs_sb": hs_sb, "x_sb": x_sb, "t0": t0,
                        "y_sb": None, "n": 0,
                        "final": it == NT - 1}

            emit_expand_chunks(prev, NC_CH)
    return nc


def _rank_mask_np(S):
    s_abs = np.abs(S)
    cum = np.cumsum(s_abs) / max(float(s_abs.sum()), 1e-8)
    hit = cum >= ENERGY_THRESHOLD
    r = int(np.argmax(hit)) + 1 if hit.any() else S.shape[0]
    return (np.arange(S.shape[0]) < r).astype(S.dtype)


def _permute_dmaj(xs):
    """[1024, 4096] token-major -> d-major group layout, same shape.

    out[g*128 + p, j*128 + c] = xs[g*128 + c, j*128 + p]
    """
    xr = xs.reshape(NT, PT, KB, 128)          # [g, c, j, p]
    return np.ascontiguousarray(
        xr.transpose(0, 3, 2, 1)).reshape(SHARD, D)


def _unpermute_dmaj(ys):
    """Inverse of _permute_dmaj (it is an involution up to axis names)."""
    yr = ys.reshape(NT, 128, KB, PT)          # [g, p, j, c]
    return np.ascontiguousarray(
        yr.transpose(0, 3, 2, 1)).reshape(SHARD, D)


def make_in_maps(x, U, S, V, norm_weight, gamma):
    S = np.asarray(S, dtype=np.float32)
    keep = _rank_mask_np(S)
    U2 = (np.asarray(norm_weight, dtype=np.float32)[:, None]
          * np.asarray(U, dtype=np.float32)
          * (S * keep)[None, :]).astype(NP_BF16)
    U2p = np.zeros((D, RP), dtype=NP_BF16)
    U2p[:, :R] = U2
    U2p = np.ascontiguousarray(
        U2p.reshape(KB, 128, RP).transpose(1, 0, 2).reshape(128, KB * RP))
    V2 = (np.asarray(V, dtype=np.float32)
          * np.asarray(gamma, dtype=np.float32)[None, :]).astype(NP_BF16)
    V2r = np.zeros((128, D), dtype=NP_BF16)
    for c in range(NSTRIP):
        V2r[32 * c:32 * c + R, :] = V2
    xf = np.ascontiguousarray(
        np.asarray(x, dtype=np.float32).reshape(TOK, D)).astype(NP_BF16)
    shards = [_permute_dmaj(s) for s in np.split(xf, N_CORES, axis=0)]
    return [{"x": s, "u": U2p, "v": V2r} for s in shards]


_CACHED_NC = None


def run(x, U, S, V, norm_weight, gamma, trace=False, **kw):
    global _CACHED_NC
    if _CACHED_NC is None:
        _CACHED_NC = build_nc()
    in_maps = make_in_maps(x, U, S, V, norm_weight, gamma)
    res = run_bass_kernel_spmd(_CACHED_NC, in_maps,
                               core_ids=list(range(N_CORES)), trace=trace,
                               **kw)
    outs = [_unpermute_dmaj(np.asarray(res.results[i]["out"]))
            for i in range(N_CORES)]
    y = np.concatenate(outs, axis=0).reshape(B, T, D).astype(np.float32)
    return y, res


def kernel(x, U, S, V, norm_weight, gamma):
    y, _ = run(x, U, S, V, norm_weight, gamma, trace=False)
    return y
